# revision 16
# baseline (speedup 1.0000x reference)
"""WLS log-linear DTI FA kernel for 8 Trainium2 NeuronCores.

Reference computation (per voxel v of a 100^3 volume, 64 gradient dirs):
    s      = ln(max(dwi[v], min_diffusivity))          [64]
    fit    = design_matrix_inv[:6] @ s                 [6]
    T      = sym3x3(fit) (+ tiny SymEig noise)
    eig    = eigenvalues(T) clamped to min_diffusivity
    fa[v]  = sqrt(0.5*sum (ei-ej)^2 / sum ei^2) * mask[v]

Kernel strategy (data-parallel over voxels, 8 cores). The stream is
HBM-bandwidth-bound (~322 GB/s/core under 8-core load), so the design
keeps the input DMA running at the ceiling start-to-finish:
  - FA from tensor invariants: FA = sqrt(1.5*p2/(p2 + tr^2/3)), matching
    the eigen-clamped reference to ~5.5e-3 rel-L2 on this distribution.
  - host pre-transposes dwi to device layout (grads on partitions, two
    voxels per 128-partition column) and pre-clamps to min_diffusivity.
  - the whole 16 MB/core input lives in ONE persistent SBUF buffer
    (122 KB/partition); ALL input DMAs are issued upfront on the sync
    HWDGE queue so the 16 SDMA engines stream at the HBM ceiling with no
    compute back-pressure.
  - the log pass is split across TWO engines so neither ever paces the
    stream: ACT runs table-Ln on 12 of 16 tiles; the other 4 tiles go to
    DVE via a fused custom op (exponent via round(bits/128-.499) to int16
    + one 8-stage op computing log2 = e-127 + f + a*f*(1-f) from the raw
    bf16 bits; weights for those chunks are pre-scaled by ln2).
  - per chunk: one [128x128] x [128x14] matmul; the 14 moving columns are
    per-voxel-half [aa, bb, cc, sqrt2*d, sqrt2*e, sqrt2*f, tr] so p2 is a
    single strided 6-wide tensor_reduce over the squares.
  - squares double as the PSUM evac (custom single-pass DVE op), then an
    all-DVE per-group chain computes fa = sqrt(1.5*p2/den) with fused
    bit-trick reciprocal and Newton sqrt (as v1). Group boundaries are
    arranged so only a tiny 3-chunk chain trails the last Ln.
Host: shard/pad/pack dwi, gather/unpermute FA, multiply by mask.
"""
import sys
import types

import numpy as np
import ml_dtypes

import concourse.bass as bass
import concourse.mybir as mybir
import concourse.tile as tile
from concourse import bacc, bass_utils


def _ensure_ntff_hook():
    """bass_utils imports antenv.axon_hooks when tracing; some images lack it.
    Register a shim backed by the axon .so so NTFF profiling works (or a no-op
    getter so runs degrade to trace-less instead of crashing)."""
    try:
        import antenv.axon_hooks  # noqa: F401
        return
    except ImportError:
        pass
    try:
        from trn_agent_boot.trn_boot import _ntff_profile_via_ctypes

        hook = _ntff_profile_via_ctypes("/opt/axon/libaxon_pjrt.so")
    except Exception:
        hook = None
    mod = types.ModuleType("antenv.axon_hooks")
    mod._hook = hook
    mod.get_axon_ntff_profile_hook = lambda: mod._hook
    mod.set_axon_ntff_profile_hook = lambda h: setattr(mod, "_hook", h)
    sys.modules["antenv.axon_hooks"] = mod
    try:
        import antenv

        antenv.axon_hooks = mod
    except ImportError:
        pass


_ensure_ntff_hook()

AFT = mybir.ActivationFunctionType
ALU = mybir.AluOpType
BF16_NP = np.dtype(ml_dtypes.bfloat16)

# ---- fixed problem geometry (hardcoded per contract) ----
NX = NY = NZ = 100
V_TOT = NX * NY * NZ            # 1,000,000 voxels
G = 64                          # gradient directions
NCORES = 8
V_CORE = V_TOT // NCORES        # 125,000 voxels per core

P = 128                         # SBUF partitions
NCH = 489                       # global matmul chunks of 128 pair-columns
NPAIR = NCH * 128               # 62,592 voxel-pair columns
V_PAD = NPAIR * 2               # 125,184 >= V_CORE (0.15% padding)
F_FA = NCH * 2                  # voxels per partition in FA phase
K = 14                          # matmul outputs per pair-column (7 per voxel)
# compute tiles in chunks; tail graduated so only a 3-chunk chain trails
CH_TILES = [35] * 12 + [27, 27, 12, 3]
assert sum(CH_TILES) == NCH
T_TILES = len(CH_TILES)
CH_LO = [sum(CH_TILES[:i]) for i in range(T_TILES)]  # chunk offset per tile
# whole-tile engine split (fine-grained same-span splits trigger ~20%
# cross-engine sync overhead on every op - measured). DVE tiles are spread
# so each lands in ACT's natural catch-up gap (ACT consumes 3.95us/tile vs
# 3.40us arrival), and the end runts are split between engines.
DVE_TILES = {1, 4, 7, 10, 14, 15}
ACT_SQ = {0, 2, 3, 5, 6, 8}     # squares evac'd on ACT in its catch-up gaps
LOG_ALPHA = 0.335               # quadratic log2(1+f) ~ f + a*f*(1-f)

SQRT_MAGIC = 0x5F3759DF         # fast inverse-sqrt seed constant


# ------------------------------------------------------------------
# custom DVE ops (registered into concourse.dve_ops at import time)
# ------------------------------------------------------------------
def _register_dve_ops():
    """Register the fused ops. Single-pass DVE instructions:
      FA_SQ  : out = sq(Src0)                           (PSUM evac + square)
      FA_DEN : out = max(Src0 + Src1*C0, C1)            (den = p2 + tr2/3)
      FA_NR1 : out = Src1*(C0 - Src0*sq(Src1)*C1)       (rsqrt Newton step)
      FA_NR2M: out = Src0*Src1*(C0 - Src0*sq(Src1)*C1)  (step + mult by x)
      FA_SEED: out = C0 + Src0*C1 -> int out            (magic rsqrt seed)
      LOG2F  : f = Src0*C0 - Src1;
               out = Src1 + C1 + f + f*C2*(1 - f)       (fast log2 from bits)
    uops_sha is computed here (self-consistent pin) rather than hardcoded."""
    if "dve_ops" in _cache:
        return _cache["dve_ops"]
    import concourse.dve_ops as dve_ops
    from concourse.dve_ops import DveOp, OPS, CUSTOM_DVE_SPECS, _SUB_OPCODE_FOR_NAME
    from concourse.dve_spec import Spec, Src0, Src1, C0, C1, C2, One, sq, maxx, lower
    from concourse.dve_spec import _has_src1 as has_src1
    from concourse.dve_uop import DveOpSpec

    _f = Src0 * C0 - Src1
    defs = [
        (
            "FA_SQ",
            Spec(
                body=sq(Src0),
                reference=lambda in0, in1, c0, c1, c2: in0.astype(np.float32) ** 2,
            ),
        ),
        (
            "FA_DEN",
            Spec(
                body=maxx(Src0 + Src1 * C0, C1),
                reference=lambda in0, in1, c0, c1, c2: np.maximum(
                    in0.astype(np.float32) + in1 * c0, c1
                ),
            ),
        ),
        (
            "FA_NR1",
            Spec(
                body=Src1 * (C0 - Src0 * sq(Src1) * C1),
                reference=lambda in0, in1, c0, c1, c2: in1
                * (c0 - in0.astype(np.float32) * in1 * in1 * c1),
            ),
        ),
        (
            "FA_NR2M",
            Spec(
                body=(Src0 * Src1) * (C0 - Src0 * sq(Src1) * C1),
                reference=lambda in0, in1, c0, c1, c2: in0.astype(np.float32)
                * in1
                * (c0 - in0 * in1 * in1 * c1),
            ),
        ),
        (
            "FA_SEED",
            Spec(
                body=C0 + Src0 * C1,
                reference=lambda in0, in1, c0, c1, c2: c0
                + in0.astype(np.float32) * c1,
            ),
        ),
        (
            "LOG2F",
            Spec(
                body=Src1 + C1 + _f + _f * C2 * (One - _f),
                reference=lambda in0, in1, c0, c1, c2: (
                    lambda f: in1.astype(np.float32) + c1 + f + f * c2 * (1.0 - f)
                )(in0.astype(np.float32) * c0 - in1.astype(np.float32)),
            ),
        ),
    ]
    handles = {}
    for name, spec in defs:
        if name in _SUB_OPCODE_FOR_NAME:
            handles[name] = next(o for o in OPS if o.name == name)
            continue
        row = max(_SUB_OPCODE_FOR_NAME.values()) + 1
        assert row < 0x20, "custom DVE opcode rows exhausted"
        _SUB_OPCODE_FOR_NAME[name] = row
        shas = {}
        for ver in ("v3", "v4"):
            s = DveOpSpec(
                name=name, opcode=row, uops=lower(spec, ver=ver),
                rd1_en=has_src1(spec),
            ).sha(ver)
            shas[ver] = s
        op = DveOp(name, spec, subdim=False, uops_sha=shas)
        OPS.append(op)
        CUSTOM_DVE_SPECS[name] = spec
        handles[name] = op
    _cache["dve_ops"] = handles
    return handles


# ------------------------------------------------------------------
# host-side helpers
# ------------------------------------------------------------------
_cache = {}


def _voxel_map():
    """vmap[p, gch, u] = padded-shard voxel index at device position
    (partition p of chunk gch, pair-half u): v = 256*gch + 2*p + u."""
    if "vmap" in _cache:
        return _cache["vmap"]
    p = np.arange(P)[:, None, None]
    gch = np.arange(NCH)[None, :, None]
    u = np.arange(2)[None, None, :]
    vmap = 256 * gch + 2 * p + u  # [P, NCH, 2]
    _cache["vmap"] = vmap
    return vmap


def _wpat(design_matrix_inv):
    """Block-diagonal weight pattern [128, 28] bf16: cols 0:14 natural-log
    weights, cols 14:28 the same scaled by ln2 (for DVE log2 tiles).
    wpat[64*u+g, 7*u+m] = wp7[m, g] with rows [aa, bb, cc, sqrt2*d,
    sqrt2*e, sqrt2*f, tr]: deviatoric combos and the trace."""
    w6 = np.asarray(design_matrix_inv, dtype=np.float32)[:6]  # a d b e f c
    wtr = w6[0] + w6[2] + w6[5]
    r2 = np.float32(np.sqrt(2.0))
    wp7 = np.stack(
        [w6[0] - wtr / 3, w6[2] - wtr / 3, w6[5] - wtr / 3,
         r2 * w6[1], r2 * w6[3], r2 * w6[4], wtr]
    ).astype(np.float32)  # [7, 64]
    wpat = np.zeros((P, 2 * K), dtype=np.float32)
    for u in range(2):
        wpat[64 * u : 64 * u + 64, 7 * u : 7 * u + 7] = wp7.T
        wpat[64 * u : 64 * u + 64, K + 7 * u : K + 7 * u + 7] = (
            np.float32(np.log(2.0)) * wp7.T
        )
    return np.ascontiguousarray(wpat.astype(BF16_NP))


def _pack_core(shard_bf16_u16):
    """[V_PAD, 64] uint16 view of bf16 -> flat device layout [128, NPAIR]:
    dwiT[64*u+g, 128*gch+i] = dwi[(gch*128+i)*2+u, g]."""
    a = shard_bf16_u16.reshape(NCH, 128, 2, G)
    a = a.transpose(2, 3, 0, 1)  # [u, g, gch, i]
    return np.ascontiguousarray(a.reshape(P, NPAIR))


# ------------------------------------------------------------------
# device program
# ------------------------------------------------------------------
def _fa_group(nc, fat, sq_all, fa_all, fa_d, ops, lo, hi):
    """FA for chunk range [lo, hi): squares -> p2/den -> fa.
    Inputs are squares of [aa, bb, cc, sqrt2*d, sqrt2*e, sqrt2*f, tr] at
    stride 7, so p2 is one strided 6-wide tensor_reduce."""
    F = (hi - lo) * 2
    f32 = mybir.dt.float32
    i32 = mybir.dt.int32
    sq = sq_all[:, lo * K : hi * K].rearrange("p (n k) -> p n k", k=7)
    q_tr = sq[:, :, 6]

    def tl(tag, dt=f32):
        return fat.tile([P, F], dt, tag=tag, name=tag)

    p2 = tl("p2")
    den = tl("den"); z2 = tl("z2")
    sw = tl("sw", i32)

    # the whole chain stays on DVE: in-engine dependencies execute
    # back-to-back with no semaphores (cross-engine ping-pong head-blocks
    # the in-order queues)
    v = nc.vector
    v.tensor_reduce(out=p2, in_=sq[:, :, 0:6], axis=mybir.AxisListType.X, op=ALU.add)
    # den = max(p2 + tr2/3, 1e-30)
    v._custom_dve(ops["FA_DEN"], out=den, in0=p2, in1=q_tr, s0=1.0 / 3.0, s1=1e-30)
    # irec = 1/den (bit-trick + 2 Newton, one pass); z2 = fa^2 = 1.5*p2*irec
    v.reciprocal_approx_fast(out=den, in_=den)
    v.scalar_tensor_tensor(out=z2, in0=p2, scalar=1.5, in1=den, op0=ALU.mult, op1=ALU.mult)
    # sqrt(z2): magic rsqrt seed in ONE fused pass: read bits(z2) as int
    # (converted to float at the read port), affine, int-rounding write
    v._custom_dve(
        ops["FA_SEED"], out=sw, in0=z2.bitcast(i32),
        s0=float(SQRT_MAGIC), s1=-0.5,
    )
    # one Newton step suffices: 0.18% max err vs the 0.4% bf16 out rounding
    v._custom_dve(
        ops["FA_NR2M"], out=fa_all[:, lo * 2 : hi * 2], in0=z2,
        in1=sw.bitcast(f32), s0=1.5, s1=0.5
    )
    # stream this group's FA out now; only the last group's DMA is tail
    nc.sync.dma_start(
        out=fa_d[:, lo * 2 : hi * 2], in_=fa_all[:, lo * 2 : hi * 2]
    )


def _build_program(mind: float):
    ops = _register_dve_ops()
    nc = bacc.Bacc("TRN2", target_bir_lowering=False, debug=False, num_devices=NCORES)
    f32 = mybir.dt.float32
    bf16 = mybir.dt.bfloat16
    i16 = mybir.dt.int16

    dwi_d = nc.dram_tensor("dwi", [P, NPAIR], bf16, kind="ExternalInput")
    wpat_d = nc.dram_tensor("wpat", [P, 2 * K], bf16, kind="ExternalInput")
    fa_d = nc.dram_tensor("fa", [P, F_FA], bf16, kind="ExternalOutput")

    # FA group boundaries (in tiles): big early, tiny at the very end so the
    # post-last-Ln tail is one short chain over 3 chunks
    SLICES = [(0, 3), (3, 6), (6, 9), (9, 12), (12, 16)]
    slice_end = {hi: (lo, hi) for lo, hi in SLICES}
    # tile 0 is DMA'd and Ln'd in three pieces so the first Ln starts as
    # soon as the first few hundred KB land. Spans stay >=512 cols: small
    # activations pick a different act-table variant (second table load).
    W0 = CH_TILES[0] * 128
    T0_SPANS = [(0, 512), (512, 1536), (1536, W0)]  # clipped to ACT part per-tile

    with tile.TileContext(nc) as tc:
        with (
            tc.tile_pool(name="singles", bufs=1) as singles,
            tc.tile_pool(name="persist", bufs=1) as persist,
            tc.tile_pool(name="e16p", bufs=2) as e16p,
            tc.tile_pool(name="psum", bufs=8, space="PSUM") as psum_pool,
            tc.tile_pool(name="fat", bufs=2) as fat,
        ):
            consts = singles.tile([P, 2], f32, tag="consts", name="consts")
            nc.vector.memset(consts[:, 0:1], mind)
            # warmup Ln on a memset scratch: attaches the act-table load to
            # an instruction with no DMA dependency, pulling the 1.28us
            # load off the first-tile critical path
            warm = singles.tile([P, 512], bf16, tag="warm", name="warm")
            nc.vector.memset(warm, 1.0)
            nc.scalar.activation(
                out=warm, in_=warm, func=AFT.Ln, bias=consts[:, 0:1]
            )

            # whole input resident in SBUF; all input DMA issued upfront on
            # the sync hwdge queue so SDMA streams at the HBM ceiling
            dwi_sb = persist.tile([P, NPAIR], bf16, tag="dwi", name="dwi_sb")
            sq_all = persist.tile([P, NCH * K], f32, tag="sq", name="sq_all")
            fa_all = persist.tile([P, F_FA], bf16, tag="fa", name="fa_all")
            wpat_sb = singles.tile([P, 2 * K], bf16, tag="wpat", name="wpat_sb")

            spans = []
            for t in range(T_TILES):
                base = CH_LO[t] * 128
                tspans = T0_SPANS if t == 0 else [(0, CH_TILES[t] * 128)]
                spans.append(tspans)
                for a, b in tspans:
                    nc.sync.dma_start(
                        out=dwi_sb[:, base + a : base + b],
                        in_=dwi_d[:, base + a : base + b],
                    )
                if t == 0:
                    nc.gpsimd.dma_start(out=wpat_sb, in_=wpat_d[:, :])

            pend_sq = []
            pend_group = []
            for t in range(T_TILES):
                wt = CH_TILES[t] * 128
                base = CH_LO[t] * 128
                sT = dwi_sb[:, base : base + wt]
                if t in DVE_TILES:
                    # fast log2 on DVE: e = round(bits/128 - .499) (int16,
                    # hw rounds to nearest); out = e-127 + f + a*f*(1-f)
                    e16 = e16p.tile([P, wt], i16, tag="e16", name="e16")
                    nc.vector.tensor_scalar(
                        out=e16, in0=sT.bitcast(i16),
                        scalar1=1.0 / 128.0, op0=ALU.mult,
                        scalar2=-0.499, op1=ALU.add,
                    )
                    nc.vector._custom_dve(
                        ops["LOG2F"], out=sT, in0=sT.bitcast(i16), in1=e16,
                        s0=1.0 / 128.0, s1=-127.0, imm2=LOG_ALPHA,
                    )
                else:
                    for a, b in spans[t]:
                        # s = ln(dwi + mind)  (~= ln(max(dwi, mind)))
                        nc.scalar.activation(
                            out=dwi_sb[:, base + a : base + b],
                            in_=dwi_sb[:, base + a : base + b], func=AFT.Ln,
                            bias=consts[:, 0:1],
                        )
                pt = psum_pool.tile([P, CH_TILES[t] * K], f32, tag="ps", name="ps")
                woff = K if t in DVE_TILES else 0
                for c in range(CH_TILES[t]):
                    nc.tensor.matmul(
                        out=pt[:, c * K : (c + 1) * K],
                        lhsT=sT[:, c * 128 : (c + 1) * 128],
                        rhs=wpat_sb[:, woff : woff + K],
                        start=True,
                        stop=True,
                    )
                # squares are all FA needs; also the PSUM evac. ACT evacs
                # lag one tile so tile t's matmuls overlap Ln_{t+1} instead
                # of stalling the scalar queue
                sq_out = sq_all[:, CH_LO[t] * K : (CH_LO[t] + CH_TILES[t]) * K]
                if t in ACT_SQ:
                    pend_sq.append((sq_out, pt))
                    if len(pend_sq) > 1:
                        so, po = pend_sq.pop(0)
                        nc.scalar.activation(out=so, in_=po, func=AFT.Square)
                else:
                    nc.vector._custom_dve(ops["FA_SQ"], out=sq_out, in0=pt)
                # FA group chains are emitted one tile AFTER their data is
                # complete so they don't head-block the next DVE log on the
                # in-order vector queue
                for lo_t, hi_t in list(pend_group):
                    pend_group.remove((lo_t, hi_t))
                    while pend_sq:  # trace-order RAW for the chain's reads
                        so, po = pend_sq.pop(0)
                        nc.scalar.activation(out=so, in_=po, func=AFT.Square)
                    _fa_group(nc, fat, sq_all, fa_all, fa_d, ops,
                              CH_LO[lo_t],
                              CH_LO[hi_t] if hi_t < T_TILES else NCH)
                if (t + 1) in slice_end:
                    if t + 1 == T_TILES:
                        while pend_sq:
                            so, po = pend_sq.pop(0)
                            nc.scalar.activation(out=so, in_=po, func=AFT.Square)
                        lo_t, hi_t = slice_end[t + 1]
                        _fa_group(nc, fat, sq_all, fa_all, fa_d, ops,
                                  CH_LO[lo_t], NCH)
                    else:
                        pend_group.append(slice_end[t + 1])

    nc.compile()
    return nc


def _get_program(mind: float):
    key = ("prog", round(mind, 18))
    if key not in _cache:
        _cache[key] = _build_program(mind)
    return _cache[key]


# ------------------------------------------------------------------
# entry point
# ------------------------------------------------------------------
def kernel(dwi, mask, design_matrix_inv, min_diffusivity):
    dwi = np.ascontiguousarray(np.asarray(dwi, dtype=np.float32)).reshape(V_TOT, G)
    mask = np.asarray(mask, dtype=np.float32).reshape(V_TOT)
    mind = float(np.asarray(min_diffusivity))

    nc = _get_program(mind)
    wpat = _wpat(design_matrix_inv)

    # clamp (reference semantics; also keeps the DVE bit-trick log in its
    # valid domain: positive normal bf16)
    dwi_bf = np.maximum(dwi, np.float32(mind)).astype(BF16_NP).view(np.uint16)
    in_maps = []
    for core in range(NCORES):
        pad = np.empty((V_PAD, G), dtype=np.uint16)
        pad[:V_CORE] = dwi_bf[core * V_CORE : (core + 1) * V_CORE]
        pad[V_CORE:] = np.float32(1.0).astype(BF16_NP).view(np.uint16)
        in_maps.append(
            {"dwi": _pack_core(pad).view(BF16_NP), "wpat": wpat}
        )

    res = None
    for attempt in range(3):
        try:
            res = bass_utils.run_bass_kernel_spmd(nc, in_maps, core_ids=list(range(NCORES)))
            break
        except Exception:
            if attempt == 2:
                raise
    _cache["last_result"] = res  # exec_time_ns etc. for the dev harness

    vmap_flat = _voxel_map().reshape(-1)  # [P*NCH*2]
    fa = np.empty(V_TOT, dtype=np.float32)
    for core in range(NCORES):
        fa_dev = np.asarray(res.results[core]["fa"]).astype(np.float32).reshape(-1)
        fa_pad = np.empty(V_PAD, dtype=np.float32)
        fa_pad[vmap_flat] = fa_dev
        fa[core * V_CORE : (core + 1) * V_CORE] = fa_pad[:V_CORE]

    fa *= mask
    return fa.reshape(NX, NY, NZ, 1)


# revision 19
# speedup vs baseline: 1.0457x; 1.0457x over previous
"""WLS log-linear DTI FA kernel for 8 Trainium2 NeuronCores.

Reference computation (per voxel v of a 100^3 volume, 64 gradient dirs):
    s      = ln(max(dwi[v], min_diffusivity))          [64]
    fit    = design_matrix_inv[:6] @ s                 [6]
    T      = sym3x3(fit) (+ tiny SymEig noise)
    eig    = eigenvalues(T) clamped to min_diffusivity
    fa[v]  = sqrt(0.5*sum (ei-ej)^2 / sum ei^2) * mask[v]

Kernel strategy (data-parallel over voxels, 8 cores). The stream is
HBM-bandwidth-bound (~322 GB/s/core under 8-core load), so the design
keeps the input DMA running at the ceiling start-to-finish:
  - FA from tensor invariants: FA = sqrt(1.5*p2/(p2 + tr^2/3)), matching
    the eigen-clamped reference to ~5.5e-3 rel-L2 on this distribution.
  - host pre-transposes dwi to device layout (grads on partitions, two
    voxels per 128-partition column) and pre-clamps to min_diffusivity.
  - the whole 16 MB/core input lives in ONE persistent SBUF buffer
    (122 KB/partition); ALL input DMAs are issued upfront on the sync
    HWDGE queue so the 16 SDMA engines stream at the HBM ceiling with no
    compute back-pressure.
  - the log pass is split across TWO engines so neither ever paces the
    stream: ACT runs table-Ln on 12 of 16 tiles; the other 4 tiles go to
    DVE via a fused custom op (exponent via round(bits/128-.499) to int16
    + one 8-stage op computing log2 = e-127 + f + a*f*(1-f) from the raw
    bf16 bits; weights for those chunks are pre-scaled by ln2).
  - per chunk: one [128x128] x [128x14] matmul; the 14 moving columns are
    per-voxel-half [aa, bb, cc, sqrt2*d, sqrt2*e, sqrt2*f, tr] so p2 is a
    single strided 6-wide tensor_reduce over the squares.
  - squares double as the PSUM evac (custom single-pass DVE op), then an
    all-DVE per-group chain computes fa = sqrt(1.5*p2/den) with fused
    bit-trick reciprocal and Newton sqrt (as v1). Group boundaries are
    arranged so only a tiny 3-chunk chain trails the last Ln.
Host: shard/pad/pack dwi, gather/unpermute FA, multiply by mask.
"""
import sys
import types

import numpy as np
import ml_dtypes

import concourse.bass as bass
import concourse.mybir as mybir
import concourse.tile as tile
from concourse import bacc, bass_utils


def _ensure_ntff_hook():
    """bass_utils imports antenv.axon_hooks when tracing; some images lack it.
    Register a shim backed by the axon .so so NTFF profiling works (or a no-op
    getter so runs degrade to trace-less instead of crashing)."""
    try:
        import antenv.axon_hooks  # noqa: F401
        return
    except ImportError:
        pass
    try:
        from trn_agent_boot.trn_boot import _ntff_profile_via_ctypes

        hook = _ntff_profile_via_ctypes("/opt/axon/libaxon_pjrt.so")
    except Exception:
        hook = None
    mod = types.ModuleType("antenv.axon_hooks")
    mod._hook = hook
    mod.get_axon_ntff_profile_hook = lambda: mod._hook
    mod.set_axon_ntff_profile_hook = lambda h: setattr(mod, "_hook", h)
    sys.modules["antenv.axon_hooks"] = mod
    try:
        import antenv

        antenv.axon_hooks = mod
    except ImportError:
        pass


_ensure_ntff_hook()

AFT = mybir.ActivationFunctionType
ALU = mybir.AluOpType
BF16_NP = np.dtype(ml_dtypes.bfloat16)

# ---- fixed problem geometry (hardcoded per contract) ----
NX = NY = NZ = 100
V_TOT = NX * NY * NZ            # 1,000,000 voxels
G = 64                          # gradient directions
NCORES = 8
V_CORE = V_TOT // NCORES        # 125,000 voxels per core

P = 128                         # SBUF partitions
NCH = 489                       # global matmul chunks of 128 pair-columns
NPAIR = NCH * 128               # 62,592 voxel-pair columns
V_PAD = NPAIR * 2               # 125,184 >= V_CORE (0.15% padding)
F_FA = NCH * 2                  # voxels per partition in FA phase
K = 14                          # matmul outputs per pair-column (7 per voxel)
# compute tiles in chunks; tail graduated so only a 3-chunk chain trails
CH_TILES = [35] * 12 + [27, 27, 12, 3]
assert sum(CH_TILES) == NCH
T_TILES = len(CH_TILES)
CH_LO = [sum(CH_TILES[:i]) for i in range(T_TILES)]  # chunk offset per tile
# whole-tile engine split (fine-grained same-span splits trigger ~20%
# cross-engine sync overhead on every op - measured). DVE tiles are spread
# so each lands in ACT's natural catch-up gap (ACT consumes 3.95us/tile vs
# 3.40us arrival), and the end runts are split between engines.
DVE_TILES = {1, 4, 7, 10}
ACT_SQ = {0, 2, 3, 5, 6, 8}     # squares evac'd on ACT in its catch-up gaps
LOG_ALPHA = 0.335               # quadratic log2(1+f) ~ f + a*f*(1-f)

SQRT_MAGIC = 0x5F3759DF         # fast inverse-sqrt seed constant


# ------------------------------------------------------------------
# custom DVE ops (registered into concourse.dve_ops at import time)
# ------------------------------------------------------------------
def _register_dve_ops():
    """Register the fused ops. Single-pass DVE instructions:
      FA_SQ  : out = sq(Src0)                           (PSUM evac + square)
      FA_DEN : out = max(Src0 + Src1*C0, C1)            (den = p2 + tr2/3)
      FA_NR1 : out = Src1*(C0 - Src0*sq(Src1)*C1)       (rsqrt Newton step)
      FA_NR2M: out = Src0*Src1*(C0 - Src0*sq(Src1)*C1)  (step + mult by x)
      FA_SEED: out = C0 + Src0*C1 -> int out            (magic rsqrt seed)
      LOG2F  : f = Src0*C0 - Src1;
               out = Src1 + C1 + f + f*C2*(1 - f)       (fast log2 from bits)
    uops_sha is computed here (self-consistent pin) rather than hardcoded."""
    if "dve_ops" in _cache:
        return _cache["dve_ops"]
    import concourse.dve_ops as dve_ops
    from concourse.dve_ops import DveOp, OPS, CUSTOM_DVE_SPECS, _SUB_OPCODE_FOR_NAME
    from concourse.dve_spec import Spec, Src0, Src1, C0, C1, C2, One, sq, maxx, lower
    from concourse.dve_spec import _has_src1 as has_src1
    from concourse.dve_uop import DveOpSpec

    _f = Src0 * C0 - Src1
    defs = [
        (
            "FA_SQ",
            Spec(
                body=sq(Src0),
                reference=lambda in0, in1, c0, c1, c2: in0.astype(np.float32) ** 2,
            ),
        ),
        (
            "FA_DEN",
            Spec(
                body=maxx(Src0 + Src1 * C0, C1),
                reference=lambda in0, in1, c0, c1, c2: np.maximum(
                    in0.astype(np.float32) + in1 * c0, c1
                ),
            ),
        ),
        (
            "FA_NR1",
            Spec(
                body=Src1 * (C0 - Src0 * sq(Src1) * C1),
                reference=lambda in0, in1, c0, c1, c2: in1
                * (c0 - in0.astype(np.float32) * in1 * in1 * c1),
            ),
        ),
        (
            "FA_NR2M",
            Spec(
                body=(Src0 * Src1) * (C0 - Src0 * sq(Src1) * C1),
                reference=lambda in0, in1, c0, c1, c2: in0.astype(np.float32)
                * in1
                * (c0 - in0 * in1 * in1 * c1),
            ),
        ),
        (
            "FA_SEED",
            Spec(
                body=C0 + Src0 * C1,
                reference=lambda in0, in1, c0, c1, c2: c0
                + in0.astype(np.float32) * c1,
            ),
        ),
        (
            "LOG2F",
            Spec(
                body=Src1 + C1 + _f + _f * C2 * (One - _f),
                reference=lambda in0, in1, c0, c1, c2: (
                    lambda f: in1.astype(np.float32) + c1 + f + f * c2 * (1.0 - f)
                )(in0.astype(np.float32) * c0 - in1.astype(np.float32)),
            ),
        ),
    ]
    handles = {}
    for name, spec in defs:
        if name in _SUB_OPCODE_FOR_NAME:
            handles[name] = next(o for o in OPS if o.name == name)
            continue
        row = max(_SUB_OPCODE_FOR_NAME.values()) + 1
        assert row < 0x20, "custom DVE opcode rows exhausted"
        _SUB_OPCODE_FOR_NAME[name] = row
        shas = {}
        for ver in ("v3", "v4"):
            s = DveOpSpec(
                name=name, opcode=row, uops=lower(spec, ver=ver),
                rd1_en=has_src1(spec),
            ).sha(ver)
            shas[ver] = s
        op = DveOp(name, spec, subdim=False, uops_sha=shas)
        OPS.append(op)
        CUSTOM_DVE_SPECS[name] = spec
        handles[name] = op
    _cache["dve_ops"] = handles
    return handles


# ------------------------------------------------------------------
# host-side helpers
# ------------------------------------------------------------------
_cache = {}


def _voxel_map():
    """vmap[p, gch, u] = padded-shard voxel index at device position
    (partition p of chunk gch, pair-half u): v = 256*gch + 2*p + u."""
    if "vmap" in _cache:
        return _cache["vmap"]
    p = np.arange(P)[:, None, None]
    gch = np.arange(NCH)[None, :, None]
    u = np.arange(2)[None, None, :]
    vmap = 256 * gch + 2 * p + u  # [P, NCH, 2]
    _cache["vmap"] = vmap
    return vmap


def _wpat(design_matrix_inv):
    """Block-diagonal weight pattern [128, 28] bf16: cols 0:14 natural-log
    weights, cols 14:28 the same scaled by ln2 (for DVE log2 tiles).
    wpat[64*u+g, 7*u+m] = wp7[m, g] with rows [aa, bb, cc, sqrt2*d,
    sqrt2*e, sqrt2*f, tr]: deviatoric combos and the trace."""
    w6 = np.asarray(design_matrix_inv, dtype=np.float32)[:6]  # a d b e f c
    wtr = w6[0] + w6[2] + w6[5]
    r2 = np.float32(np.sqrt(2.0))
    wp7 = np.stack(
        [w6[0] - wtr / 3, w6[2] - wtr / 3, w6[5] - wtr / 3,
         r2 * w6[1], r2 * w6[3], r2 * w6[4], wtr]
    ).astype(np.float32)  # [7, 64]
    wpat = np.zeros((P, 2 * K), dtype=np.float32)
    for u in range(2):
        wpat[64 * u : 64 * u + 64, 7 * u : 7 * u + 7] = wp7.T
        wpat[64 * u : 64 * u + 64, K + 7 * u : K + 7 * u + 7] = (
            np.float32(np.log(2.0)) * wp7.T
        )
    return np.ascontiguousarray(wpat.astype(BF16_NP))


def _pack_core(shard_bf16_u16):
    """[V_PAD, 64] uint16 view of bf16 -> flat device layout [128, NPAIR]:
    dwiT[64*u+g, 128*gch+i] = dwi[(gch*128+i)*2+u, g]."""
    a = shard_bf16_u16.reshape(NCH, 128, 2, G)
    a = a.transpose(2, 3, 0, 1)  # [u, g, gch, i]
    return np.ascontiguousarray(a.reshape(P, NPAIR))


# ------------------------------------------------------------------
# device program
# ------------------------------------------------------------------
def _fa_group(nc, fat, sq_all, fa_all, fa_d, ops, lo, hi):
    """FA for chunk range [lo, hi): squares -> p2/den -> fa.
    Inputs are squares of [aa, bb, cc, sqrt2*d, sqrt2*e, sqrt2*f, tr] at
    stride 7, so p2 is one strided 6-wide tensor_reduce."""
    F = (hi - lo) * 2
    f32 = mybir.dt.float32
    i32 = mybir.dt.int32
    sq = sq_all[:, lo * K : hi * K].rearrange("p (n k) -> p n k", k=7)
    q_tr = sq[:, :, 6]

    def tl(tag, dt=f32):
        return fat.tile([P, F], dt, tag=tag, name=tag)

    p2 = tl("p2")
    den = tl("den"); z2 = tl("z2")
    sw = tl("sw", i32)

    # the whole chain stays on DVE: in-engine dependencies execute
    # back-to-back with no semaphores (cross-engine ping-pong head-blocks
    # the in-order queues)
    v = nc.vector
    v.tensor_reduce(out=p2, in_=sq[:, :, 0:6], axis=mybir.AxisListType.X, op=ALU.add)
    # den = max(p2 + tr2/3, 1e-30)
    v._custom_dve(ops["FA_DEN"], out=den, in0=p2, in1=q_tr, s0=1.0 / 3.0, s1=1e-30)
    # irec = 1/den (bit-trick + 2 Newton, one pass); z2 = fa^2 = 1.5*p2*irec
    v.reciprocal_approx_fast(out=den, in_=den)
    v.scalar_tensor_tensor(out=z2, in0=p2, scalar=1.5, in1=den, op0=ALU.mult, op1=ALU.mult)
    # sqrt(z2): magic rsqrt seed in ONE fused pass: read bits(z2) as int
    # (converted to float at the read port), affine, int-rounding write
    v._custom_dve(
        ops["FA_SEED"], out=sw, in0=z2.bitcast(i32),
        s0=float(SQRT_MAGIC), s1=-0.5,
    )
    # one Newton step suffices: 0.18% max err vs the 0.4% bf16 out rounding
    v._custom_dve(
        ops["FA_NR2M"], out=fa_all[:, lo * 2 : hi * 2], in0=z2,
        in1=sw.bitcast(f32), s0=1.5, s1=0.5
    )
    # stream this group's FA out now; only the last group's DMA is tail
    nc.sync.dma_start(
        out=fa_d[:, lo * 2 : hi * 2], in_=fa_all[:, lo * 2 : hi * 2]
    )


def _build_program(mind: float):
    ops = _register_dve_ops()
    nc = bacc.Bacc("TRN2", target_bir_lowering=False, debug=False, num_devices=NCORES)
    f32 = mybir.dt.float32
    bf16 = mybir.dt.bfloat16
    i16 = mybir.dt.int16

    dwi_d = nc.dram_tensor("dwi", [P, NPAIR], bf16, kind="ExternalInput")
    wpat_d = nc.dram_tensor("wpat", [P, 2 * K], bf16, kind="ExternalInput")
    fa_d = nc.dram_tensor("fa", [P, F_FA], bf16, kind="ExternalOutput")

    # FA group boundaries (in tiles): big early, tiny at the very end so the
    # post-last-Ln tail is one short chain over 3 chunks
    SLICES = [(0, 3), (3, 6), (6, 9), (9, 12), (12, 15), (15, 16)]
    slice_end = {hi: (lo, hi) for lo, hi in SLICES}
    # tile 0 is DMA'd and Ln'd in three pieces so the first Ln starts as
    # soon as the first few hundred KB land. Spans stay >=512 cols: small
    # activations pick a different act-table variant (second table load).
    W0 = CH_TILES[0] * 128
    T0_SPANS = [(0, 512), (512, 1536), (1536, 3200)]
    T0_DVE = (3200, W0)  # 10-chunk t0 tail fast-logged on DVE

    with tile.TileContext(nc) as tc:
        with (
            tc.tile_pool(name="singles", bufs=1) as singles,
            tc.tile_pool(name="persist", bufs=1) as persist,
            tc.tile_pool(name="e16p", bufs=2) as e16p,
            tc.tile_pool(name="psum", bufs=8, space="PSUM") as psum_pool,
            tc.tile_pool(name="fat", bufs=2) as fat,
        ):
            consts = singles.tile([P, 2], f32, tag="consts", name="consts")
            nc.vector.memset(consts[:, 0:1], mind)
            # warmup Ln on a memset scratch: attaches the act-table load to
            # an instruction with no DMA dependency, pulling the 1.28us
            # load off the first-tile critical path
            warm = singles.tile([P, 512], bf16, tag="warm", name="warm")
            nc.vector.memset(warm, 1.0)
            nc.scalar.activation(
                out=warm, in_=warm, func=AFT.Ln, bias=consts[:, 0:1]
            )

            # whole input resident in SBUF; all input DMA issued upfront on
            # the sync hwdge queue so SDMA streams at the HBM ceiling
            dwi_sb = persist.tile([P, NPAIR], bf16, tag="dwi", name="dwi_sb")
            sq_all = persist.tile([P, NCH * K], f32, tag="sq", name="sq_all")
            fa_all = persist.tile([P, F_FA], bf16, tag="fa", name="fa_all")
            wpat_sb = singles.tile([P, 2 * K], bf16, tag="wpat", name="wpat_sb")

            spans = []
            for t in range(T_TILES):
                base = CH_LO[t] * 128
                tspans = T0_SPANS if t == 0 else [(0, CH_TILES[t] * 128)]
                spans.append(tspans)
                dspans = (T0_SPANS[:-1] + [(1536, W0)]) if t == 0 else tspans
                for a, b in dspans:
                    nc.sync.dma_start(
                        out=dwi_sb[:, base + a : base + b],
                        in_=dwi_d[:, base + a : base + b],
                    )
                if t == 0:
                    nc.gpsimd.dma_start(out=wpat_sb, in_=wpat_d[:, :])

            pend_sq = []
            pend_group = []
            for t in range(T_TILES):
                wt = CH_TILES[t] * 128
                base = CH_LO[t] * 128
                sT = dwi_sb[:, base : base + wt]
                if t == 0:
                    a, b = T0_DVE
                    e16 = e16p.tile([P, 4480], i16, tag="e16", name="e16")[:, : b - a]
                    nc.vector.tensor_scalar(
                        out=e16, in0=sT[:, a:b].bitcast(i16),
                        scalar1=1.0 / 128.0, op0=ALU.mult,
                        scalar2=-0.499, op1=ALU.add,
                    )
                    nc.vector._custom_dve(
                        ops["LOG2F"], out=sT[:, a:b], in0=sT[:, a:b].bitcast(i16),
                        in1=e16, s0=1.0 / 128.0, s1=-127.0, imm2=LOG_ALPHA,
                    )
                if t in DVE_TILES:
                    # fast log2 on DVE: e = round(bits/128 - .499) (int16,
                    # hw rounds to nearest); out = e-127 + f + a*f*(1-f)
                    e16 = e16p.tile([P, 4480], i16, tag="e16", name="e16")[:, :wt]
                    nc.vector.tensor_scalar(
                        out=e16, in0=sT.bitcast(i16),
                        scalar1=1.0 / 128.0, op0=ALU.mult,
                        scalar2=-0.499, op1=ALU.add,
                    )
                    nc.vector._custom_dve(
                        ops["LOG2F"], out=sT, in0=sT.bitcast(i16), in1=e16,
                        s0=1.0 / 128.0, s1=-127.0, imm2=LOG_ALPHA,
                    )
                else:
                    for a, b in spans[t]:
                        # s = ln(dwi + mind)  (~= ln(max(dwi, mind)))
                        nc.scalar.activation(
                            out=dwi_sb[:, base + a : base + b],
                            in_=dwi_sb[:, base + a : base + b], func=AFT.Ln,
                            bias=consts[:, 0:1],
                        )
                pt = psum_pool.tile([P, CH_TILES[t] * K], f32, tag="ps", name="ps")
                for c in range(CH_TILES[t]):
                    woff = K if (t in DVE_TILES or (t == 0 and c >= 25)) else 0
                    nc.tensor.matmul(
                        out=pt[:, c * K : (c + 1) * K],
                        lhsT=sT[:, c * 128 : (c + 1) * 128],
                        rhs=wpat_sb[:, woff : woff + K],
                        start=True,
                        stop=True,
                    )
                # squares are all FA needs; also the PSUM evac. ACT evacs
                # lag one tile so tile t's matmuls overlap Ln_{t+1} instead
                # of stalling the scalar queue
                sq_out = sq_all[:, CH_LO[t] * K : (CH_LO[t] + CH_TILES[t]) * K]
                if t in ACT_SQ:
                    pend_sq.append((sq_out, pt))
                    if len(pend_sq) > 1:
                        so, po = pend_sq.pop(0)
                        nc.scalar.activation(out=so, in_=po, func=AFT.Square)
                else:
                    nc.vector._custom_dve(ops["FA_SQ"], out=sq_out, in0=pt)
                # FA group chains are emitted one tile AFTER their data is
                # complete so they don't head-block the next DVE log on the
                # in-order vector queue
                for lo_t, hi_t in list(pend_group):
                    pend_group.remove((lo_t, hi_t))
                    while pend_sq:  # trace-order RAW for the chain's reads
                        so, po = pend_sq.pop(0)
                        nc.scalar.activation(out=so, in_=po, func=AFT.Square)
                    _fa_group(nc, fat, sq_all, fa_all, fa_d, ops,
                              CH_LO[lo_t],
                              CH_LO[hi_t] if hi_t < T_TILES else NCH)
                if (t + 1) in slice_end:
                    if t + 1 == T_TILES:
                        while pend_sq:
                            so, po = pend_sq.pop(0)
                            nc.scalar.activation(out=so, in_=po, func=AFT.Square)
                        lo_t, hi_t = slice_end[t + 1]
                        _fa_group(nc, fat, sq_all, fa_all, fa_d, ops,
                                  CH_LO[lo_t], NCH)
                    else:
                        pend_group.append(slice_end[t + 1])

    nc.compile()
    return nc


def _get_program(mind: float):
    key = ("prog", round(mind, 18))
    if key not in _cache:
        _cache[key] = _build_program(mind)
    return _cache[key]


# ------------------------------------------------------------------
# entry point
# ------------------------------------------------------------------
def kernel(dwi, mask, design_matrix_inv, min_diffusivity):
    dwi = np.ascontiguousarray(np.asarray(dwi, dtype=np.float32)).reshape(V_TOT, G)
    mask = np.asarray(mask, dtype=np.float32).reshape(V_TOT)
    mind = float(np.asarray(min_diffusivity))

    nc = _get_program(mind)
    wpat = _wpat(design_matrix_inv)

    # clamp (reference semantics; also keeps the DVE bit-trick log in its
    # valid domain: positive normal bf16)
    dwi_bf = np.maximum(dwi, np.float32(mind)).astype(BF16_NP).view(np.uint16)
    in_maps = []
    for core in range(NCORES):
        pad = np.empty((V_PAD, G), dtype=np.uint16)
        pad[:V_CORE] = dwi_bf[core * V_CORE : (core + 1) * V_CORE]
        pad[V_CORE:] = np.float32(1.0).astype(BF16_NP).view(np.uint16)
        in_maps.append(
            {"dwi": _pack_core(pad).view(BF16_NP), "wpat": wpat}
        )

    res = None
    for attempt in range(3):
        try:
            res = bass_utils.run_bass_kernel_spmd(nc, in_maps, core_ids=list(range(NCORES)))
            break
        except Exception:
            if attempt == 2:
                raise
    _cache["last_result"] = res  # exec_time_ns etc. for the dev harness

    vmap_flat = _voxel_map().reshape(-1)  # [P*NCH*2]
    fa = np.empty(V_TOT, dtype=np.float32)
    for core in range(NCORES):
        fa_dev = np.asarray(res.results[core]["fa"]).astype(np.float32).reshape(-1)
        fa_pad = np.empty(V_PAD, dtype=np.float32)
        fa_pad[vmap_flat] = fa_dev
        fa[core * V_CORE : (core + 1) * V_CORE] = fa_pad[:V_CORE]

    fa *= mask
    return fa.reshape(NX, NY, NZ, 1)


# revision 20
# speedup vs baseline: 1.0520x; 1.0060x over previous
"""WLS log-linear DTI FA kernel for 8 Trainium2 NeuronCores.

Reference computation (per voxel v of a 100^3 volume, 64 gradient dirs):
    s      = ln(max(dwi[v], min_diffusivity))          [64]
    fit    = design_matrix_inv[:6] @ s                 [6]
    T      = sym3x3(fit) (+ tiny SymEig noise)
    eig    = eigenvalues(T) clamped to min_diffusivity
    fa[v]  = sqrt(0.5*sum (ei-ej)^2 / sum ei^2) * mask[v]

Kernel strategy (data-parallel over voxels, 8 cores). The stream is
HBM-bandwidth-bound (~322 GB/s/core under 8-core load), so the design
keeps the input DMA running at the ceiling start-to-finish:
  - FA from tensor invariants: FA = sqrt(1.5*p2/(p2 + tr^2/3)), matching
    the eigen-clamped reference to ~5.5e-3 rel-L2 on this distribution.
  - host pre-transposes dwi to device layout (grads on partitions, two
    voxels per 128-partition column) and pre-clamps to min_diffusivity.
  - the whole 16 MB/core input lives in ONE persistent SBUF buffer
    (122 KB/partition); ALL input DMAs are issued upfront on the sync
    HWDGE queue so the 16 SDMA engines stream at the HBM ceiling with no
    compute back-pressure.
  - the log pass is split across TWO engines so neither ever paces the
    stream: ACT runs table-Ln on 12 of 16 tiles; the other 4 tiles go to
    DVE via a fused custom op (exponent via round(bits/128-.499) to int16
    + one 8-stage op computing log2 = e-127 + f + a*f*(1-f) from the raw
    bf16 bits; weights for those chunks are pre-scaled by ln2).
  - per chunk: one [128x128] x [128x14] matmul; the 14 moving columns are
    per-voxel-half [aa, bb, cc, sqrt2*d, sqrt2*e, sqrt2*f, tr] so p2 is a
    single strided 6-wide tensor_reduce over the squares.
  - squares double as the PSUM evac (custom single-pass DVE op), then an
    all-DVE per-group chain computes fa = sqrt(1.5*p2/den) with fused
    bit-trick reciprocal and Newton sqrt (as v1). Group boundaries are
    arranged so only a tiny 3-chunk chain trails the last Ln.
Host: shard/pad/pack dwi, gather/unpermute FA, multiply by mask.
"""
import sys
import types

import numpy as np
import ml_dtypes

import concourse.bass as bass
import concourse.mybir as mybir
import concourse.tile as tile
from concourse import bacc, bass_utils


def _ensure_ntff_hook():
    """bass_utils imports antenv.axon_hooks when tracing; some images lack it.
    Register a shim backed by the axon .so so NTFF profiling works (or a no-op
    getter so runs degrade to trace-less instead of crashing)."""
    try:
        import antenv.axon_hooks  # noqa: F401
        return
    except ImportError:
        pass
    try:
        from trn_agent_boot.trn_boot import _ntff_profile_via_ctypes

        hook = _ntff_profile_via_ctypes("/opt/axon/libaxon_pjrt.so")
    except Exception:
        hook = None
    mod = types.ModuleType("antenv.axon_hooks")
    mod._hook = hook
    mod.get_axon_ntff_profile_hook = lambda: mod._hook
    mod.set_axon_ntff_profile_hook = lambda h: setattr(mod, "_hook", h)
    sys.modules["antenv.axon_hooks"] = mod
    try:
        import antenv

        antenv.axon_hooks = mod
    except ImportError:
        pass


_ensure_ntff_hook()

AFT = mybir.ActivationFunctionType
ALU = mybir.AluOpType
BF16_NP = np.dtype(ml_dtypes.bfloat16)

# ---- fixed problem geometry (hardcoded per contract) ----
NX = NY = NZ = 100
V_TOT = NX * NY * NZ            # 1,000,000 voxels
G = 64                          # gradient directions
NCORES = 8
V_CORE = V_TOT // NCORES        # 125,000 voxels per core

P = 128                         # SBUF partitions
NCH = 489                       # global matmul chunks of 128 pair-columns
NPAIR = NCH * 128               # 62,592 voxel-pair columns
V_PAD = NPAIR * 2               # 125,184 >= V_CORE (0.15% padding)
F_FA = NCH * 2                  # voxels per partition in FA phase
K = 14                          # matmul outputs per pair-column (7 per voxel)
# compute tiles in chunks; tail graduated so only a 3-chunk chain trails
CH_TILES = [35] * 12 + [27, 27, 12, 3]
assert sum(CH_TILES) == NCH
T_TILES = len(CH_TILES)
CH_LO = [sum(CH_TILES[:i]) for i in range(T_TILES)]  # chunk offset per tile
# whole-tile engine split (fine-grained same-span splits trigger ~20%
# cross-engine sync overhead on every op - measured). DVE tiles are spread
# so each lands in ACT's natural catch-up gap (ACT consumes 3.95us/tile vs
# 3.40us arrival), and the end runts are split between engines.
DVE_TILES = {1, 4, 7, 10}
ACT_SQ = {0, 2, 3, 5, 6, 8}     # squares evac'd on ACT in its catch-up gaps
LOG_ALPHA = 0.335               # quadratic log2(1+f) ~ f + a*f*(1-f)

SQRT_MAGIC = 0x5F3759DF         # fast inverse-sqrt seed constant


# ------------------------------------------------------------------
# custom DVE ops (registered into concourse.dve_ops at import time)
# ------------------------------------------------------------------
def _register_dve_ops():
    """Register the fused ops. Single-pass DVE instructions:
      FA_SQ  : out = sq(Src0)                           (PSUM evac + square)
      FA_DEN : out = max(Src0 + Src1*C0, C1)            (den = p2 + tr2/3)
      FA_NR1 : out = Src1*(C0 - Src0*sq(Src1)*C1)       (rsqrt Newton step)
      FA_NR2M: out = Src0*Src1*(C0 - Src0*sq(Src1)*C1)  (step + mult by x)
      FA_SEED: out = C0 + Src0*C1 -> int out            (magic rsqrt seed)
      LOG2F  : f = Src0*C0 - Src1;
               out = Src1 + C1 + f + f*C2*(1 - f)       (fast log2 from bits)
    uops_sha is computed here (self-consistent pin) rather than hardcoded."""
    if "dve_ops" in _cache:
        return _cache["dve_ops"]
    import concourse.dve_ops as dve_ops
    from concourse.dve_ops import DveOp, OPS, CUSTOM_DVE_SPECS, _SUB_OPCODE_FOR_NAME
    from concourse.dve_spec import Spec, Src0, Src1, C0, C1, C2, One, sq, maxx, lower
    from concourse.dve_spec import _has_src1 as has_src1
    from concourse.dve_uop import DveOpSpec

    _f = Src0 * C0 - Src1
    defs = [
        (
            "FA_SQ",
            Spec(
                body=sq(Src0),
                reference=lambda in0, in1, c0, c1, c2: in0.astype(np.float32) ** 2,
            ),
        ),
        (
            "FA_DEN",
            Spec(
                body=maxx(Src0 + Src1 * C0, C1),
                reference=lambda in0, in1, c0, c1, c2: np.maximum(
                    in0.astype(np.float32) + in1 * c0, c1
                ),
            ),
        ),
        (
            "FA_NR1",
            Spec(
                body=Src1 * (C0 - Src0 * sq(Src1) * C1),
                reference=lambda in0, in1, c0, c1, c2: in1
                * (c0 - in0.astype(np.float32) * in1 * in1 * c1),
            ),
        ),
        (
            "FA_NR2M",
            Spec(
                body=(Src0 * Src1) * (C0 - Src0 * sq(Src1) * C1),
                reference=lambda in0, in1, c0, c1, c2: in0.astype(np.float32)
                * in1
                * (c0 - in0 * in1 * in1 * c1),
            ),
        ),
        (
            "FA_SEED",
            Spec(
                body=C0 + Src0 * C1,
                reference=lambda in0, in1, c0, c1, c2: c0
                + in0.astype(np.float32) * c1,
            ),
        ),
        (
            "LOG2F",
            Spec(
                body=Src1 + C1 + _f + _f * C2 * (One - _f),
                reference=lambda in0, in1, c0, c1, c2: (
                    lambda f: in1.astype(np.float32) + c1 + f + f * c2 * (1.0 - f)
                )(in0.astype(np.float32) * c0 - in1.astype(np.float32)),
            ),
        ),
    ]
    handles = {}
    for name, spec in defs:
        if name in _SUB_OPCODE_FOR_NAME:
            handles[name] = next(o for o in OPS if o.name == name)
            continue
        row = max(_SUB_OPCODE_FOR_NAME.values()) + 1
        assert row < 0x20, "custom DVE opcode rows exhausted"
        _SUB_OPCODE_FOR_NAME[name] = row
        shas = {}
        for ver in ("v3", "v4"):
            s = DveOpSpec(
                name=name, opcode=row, uops=lower(spec, ver=ver),
                rd1_en=has_src1(spec),
            ).sha(ver)
            shas[ver] = s
        op = DveOp(name, spec, subdim=False, uops_sha=shas)
        OPS.append(op)
        CUSTOM_DVE_SPECS[name] = spec
        handles[name] = op
    _cache["dve_ops"] = handles
    return handles


# ------------------------------------------------------------------
# host-side helpers
# ------------------------------------------------------------------
_cache = {}


def _voxel_map():
    """vmap[p, gch, u] = padded-shard voxel index at device position
    (partition p of chunk gch, pair-half u): v = 256*gch + 2*p + u."""
    if "vmap" in _cache:
        return _cache["vmap"]
    p = np.arange(P)[:, None, None]
    gch = np.arange(NCH)[None, :, None]
    u = np.arange(2)[None, None, :]
    vmap = 256 * gch + 2 * p + u  # [P, NCH, 2]
    _cache["vmap"] = vmap
    return vmap


def _wpat(design_matrix_inv):
    """Block-diagonal weight pattern [128, 28] bf16: cols 0:14 natural-log
    weights, cols 14:28 the same scaled by ln2 (for DVE log2 tiles).
    wpat[64*u+g, 7*u+m] = wp7[m, g] with rows [aa, bb, cc, sqrt2*d,
    sqrt2*e, sqrt2*f, tr]: deviatoric combos and the trace."""
    w6 = np.asarray(design_matrix_inv, dtype=np.float32)[:6]  # a d b e f c
    wtr = w6[0] + w6[2] + w6[5]
    r2 = np.float32(np.sqrt(2.0))
    wp7 = np.stack(
        [w6[0] - wtr / 3, w6[2] - wtr / 3, w6[5] - wtr / 3,
         r2 * w6[1], r2 * w6[3], r2 * w6[4], wtr]
    ).astype(np.float32)  # [7, 64]
    wpat = np.zeros((P, 2 * K), dtype=np.float32)
    for u in range(2):
        wpat[64 * u : 64 * u + 64, 7 * u : 7 * u + 7] = wp7.T
        wpat[64 * u : 64 * u + 64, K + 7 * u : K + 7 * u + 7] = (
            np.float32(np.log(2.0)) * wp7.T
        )
    return np.ascontiguousarray(wpat.astype(BF16_NP))


def _pack_core(shard_bf16_u16):
    """[V_PAD, 64] uint16 view of bf16 -> flat device layout [128, NPAIR]:
    dwiT[64*u+g, 128*gch+i] = dwi[(gch*128+i)*2+u, g]."""
    a = shard_bf16_u16.reshape(NCH, 128, 2, G)
    a = a.transpose(2, 3, 0, 1)  # [u, g, gch, i]
    return np.ascontiguousarray(a.reshape(P, NPAIR))


# ------------------------------------------------------------------
# device program
# ------------------------------------------------------------------
def _fa_group(nc, fat, sq_all, fa_all, fa_d, ops, lo, hi):
    """FA for chunk range [lo, hi): squares -> p2/den -> fa.
    Inputs are squares of [aa, bb, cc, sqrt2*d, sqrt2*e, sqrt2*f, tr] at
    stride 7, so p2 is one strided 6-wide tensor_reduce."""
    F = (hi - lo) * 2
    f32 = mybir.dt.float32
    i32 = mybir.dt.int32
    sq = sq_all[:, lo * K : hi * K].rearrange("p (n k) -> p n k", k=7)
    q_tr = sq[:, :, 6]

    def tl(tag, dt=f32):
        return fat.tile([P, F], dt, tag=tag, name=tag)

    p2 = tl("p2")
    den = tl("den"); z2 = tl("z2")
    sw = tl("sw", i32)

    # the whole chain stays on DVE: in-engine dependencies execute
    # back-to-back with no semaphores (cross-engine ping-pong head-blocks
    # the in-order queues)
    v = nc.vector
    v.tensor_reduce(out=p2, in_=sq[:, :, 0:6], axis=mybir.AxisListType.X, op=ALU.add)
    # den = max(p2 + tr2/3, 1e-30)
    v._custom_dve(ops["FA_DEN"], out=den, in0=p2, in1=q_tr, s0=1.0 / 3.0, s1=1e-30)
    # irec = 1/den (bit-trick + 2 Newton, one pass); z2 = fa^2 = 1.5*p2*irec
    v.reciprocal_approx_fast(out=den, in_=den)
    v.scalar_tensor_tensor(out=z2, in0=p2, scalar=1.5, in1=den, op0=ALU.mult, op1=ALU.mult)
    # sqrt(z2): magic rsqrt seed in ONE fused pass: read bits(z2) as int
    # (converted to float at the read port), affine, int-rounding write
    v._custom_dve(
        ops["FA_SEED"], out=sw, in0=z2.bitcast(i32),
        s0=float(SQRT_MAGIC), s1=-0.5,
    )
    # one Newton step suffices: 0.18% max err vs the 0.4% bf16 out rounding
    v._custom_dve(
        ops["FA_NR2M"], out=fa_all[:, lo * 2 : hi * 2], in0=z2,
        in1=sw.bitcast(f32), s0=1.5, s1=0.5
    )
    # stream this group's FA out now; only the last group's DMA is tail
    nc.sync.dma_start(
        out=fa_d[:, lo * 2 : hi * 2], in_=fa_all[:, lo * 2 : hi * 2]
    )


def _build_program(mind: float):
    ops = _register_dve_ops()
    nc = bacc.Bacc("TRN2", target_bir_lowering=False, debug=False, num_devices=NCORES)
    f32 = mybir.dt.float32
    bf16 = mybir.dt.bfloat16
    i16 = mybir.dt.int16

    dwi_d = nc.dram_tensor("dwi", [P, NPAIR], bf16, kind="ExternalInput")
    wpat_d = nc.dram_tensor("wpat", [P, 2 * K], bf16, kind="ExternalInput")
    fa_d = nc.dram_tensor("fa", [P, F_FA], bf16, kind="ExternalOutput")

    # FA group boundaries (in tiles): big early, tiny at the very end so the
    # post-last-Ln tail is one short chain over 3 chunks
    SLICES = [(0, 3), (3, 6), (6, 9), (9, 12), (12, 14), (14, 16)]
    slice_end = {hi: (lo, hi) for lo, hi in SLICES}
    # tile 0 is DMA'd and Ln'd in three pieces so the first Ln starts as
    # soon as the first few hundred KB land. Spans stay >=512 cols: small
    # activations pick a different act-table variant (second table load).
    W0 = CH_TILES[0] * 128
    T0_SPANS = [(0, 512), (512, 1536), (1536, 3200)]
    T0_DVE = (3200, W0)  # 10-chunk t0 tail fast-logged on DVE

    with tile.TileContext(nc) as tc:
        with (
            tc.tile_pool(name="singles", bufs=1) as singles,
            tc.tile_pool(name="persist", bufs=1) as persist,
            tc.tile_pool(name="e16p", bufs=2) as e16p,
            tc.tile_pool(name="psum", bufs=8, space="PSUM") as psum_pool,
            tc.tile_pool(name="fat", bufs=2) as fat,
        ):
            consts = singles.tile([P, 2], f32, tag="consts", name="consts")
            nc.vector.memset(consts[:, 0:1], mind)
            # warmup Ln on a memset scratch: attaches the act-table load to
            # an instruction with no DMA dependency, pulling the 1.28us
            # load off the first-tile critical path
            warm = singles.tile([P, 512], bf16, tag="warm", name="warm")
            nc.vector.memset(warm, 1.0)
            nc.scalar.activation(
                out=warm, in_=warm, func=AFT.Ln, bias=consts[:, 0:1]
            )

            # whole input resident in SBUF; all input DMA issued upfront on
            # the sync hwdge queue so SDMA streams at the HBM ceiling
            dwi_sb = persist.tile([P, NPAIR], bf16, tag="dwi", name="dwi_sb")
            sq_all = persist.tile([P, NCH * K], f32, tag="sq", name="sq_all")
            fa_all = persist.tile([P, F_FA], bf16, tag="fa", name="fa_all")
            wpat_sb = singles.tile([P, 2 * K], bf16, tag="wpat", name="wpat_sb")

            spans = []
            for t in range(T_TILES):
                base = CH_LO[t] * 128
                tspans = T0_SPANS if t == 0 else [(0, CH_TILES[t] * 128)]
                spans.append(tspans)
                dspans = (T0_SPANS[:-1] + [(1536, W0)]) if t == 0 else tspans
                for a, b in dspans:
                    nc.sync.dma_start(
                        out=dwi_sb[:, base + a : base + b],
                        in_=dwi_d[:, base + a : base + b],
                    )
                if t == 0:
                    nc.gpsimd.dma_start(out=wpat_sb, in_=wpat_d[:, :])

            pend_sq = []
            pend_group = []
            for t in range(T_TILES):
                wt = CH_TILES[t] * 128
                base = CH_LO[t] * 128
                sT = dwi_sb[:, base : base + wt]
                if t == 0:
                    a, b = T0_DVE
                    e16 = e16p.tile([P, 4480], i16, tag="e16", name="e16")[:, : b - a]
                    nc.vector.tensor_scalar(
                        out=e16, in0=sT[:, a:b].bitcast(i16),
                        scalar1=1.0 / 128.0, op0=ALU.mult,
                        scalar2=-0.499, op1=ALU.add,
                    )
                    nc.vector._custom_dve(
                        ops["LOG2F"], out=sT[:, a:b], in0=sT[:, a:b].bitcast(i16),
                        in1=e16, s0=1.0 / 128.0, s1=-127.0, imm2=LOG_ALPHA,
                    )
                if t in DVE_TILES:
                    # fast log2 on DVE: e = round(bits/128 - .499) (int16,
                    # hw rounds to nearest); out = e-127 + f + a*f*(1-f)
                    e16 = e16p.tile([P, 4480], i16, tag="e16", name="e16")[:, :wt]
                    nc.vector.tensor_scalar(
                        out=e16, in0=sT.bitcast(i16),
                        scalar1=1.0 / 128.0, op0=ALU.mult,
                        scalar2=-0.499, op1=ALU.add,
                    )
                    nc.vector._custom_dve(
                        ops["LOG2F"], out=sT, in0=sT.bitcast(i16), in1=e16,
                        s0=1.0 / 128.0, s1=-127.0, imm2=LOG_ALPHA,
                    )
                else:
                    for a, b in spans[t]:
                        # s = ln(dwi + mind)  (~= ln(max(dwi, mind)))
                        nc.scalar.activation(
                            out=dwi_sb[:, base + a : base + b],
                            in_=dwi_sb[:, base + a : base + b], func=AFT.Ln,
                            bias=consts[:, 0:1],
                        )
                pt = psum_pool.tile([P, CH_TILES[t] * K], f32, tag="ps", name="ps")
                for c in range(CH_TILES[t]):
                    woff = K if (t in DVE_TILES or (t == 0 and c >= 25)) else 0
                    nc.tensor.matmul(
                        out=pt[:, c * K : (c + 1) * K],
                        lhsT=sT[:, c * 128 : (c + 1) * 128],
                        rhs=wpat_sb[:, woff : woff + K],
                        start=True,
                        stop=True,
                    )
                # squares are all FA needs; also the PSUM evac. ACT evacs
                # lag one tile so tile t's matmuls overlap Ln_{t+1} instead
                # of stalling the scalar queue
                sq_out = sq_all[:, CH_LO[t] * K : (CH_LO[t] + CH_TILES[t]) * K]
                if t in ACT_SQ:
                    pend_sq.append((sq_out, pt))
                    if len(pend_sq) > 1:
                        so, po = pend_sq.pop(0)
                        nc.scalar.activation(out=so, in_=po, func=AFT.Square)
                else:
                    nc.vector._custom_dve(ops["FA_SQ"], out=sq_out, in0=pt)
                # FA group chains are emitted one tile AFTER their data is
                # complete so they don't head-block the next DVE log on the
                # in-order vector queue
                for lo_t, hi_t in list(pend_group):
                    pend_group.remove((lo_t, hi_t))
                    while pend_sq:  # trace-order RAW for the chain's reads
                        so, po = pend_sq.pop(0)
                        nc.scalar.activation(out=so, in_=po, func=AFT.Square)
                    _fa_group(nc, fat, sq_all, fa_all, fa_d, ops,
                              CH_LO[lo_t],
                              CH_LO[hi_t] if hi_t < T_TILES else NCH)
                if (t + 1) in slice_end:
                    if t + 1 == T_TILES:
                        while pend_sq:
                            so, po = pend_sq.pop(0)
                            nc.scalar.activation(out=so, in_=po, func=AFT.Square)
                        lo_t, hi_t = slice_end[t + 1]
                        _fa_group(nc, fat, sq_all, fa_all, fa_d, ops,
                                  CH_LO[lo_t], NCH)
                    else:
                        pend_group.append(slice_end[t + 1])

    nc.compile()
    return nc


def _get_program(mind: float):
    key = ("prog", round(mind, 18))
    if key not in _cache:
        _cache[key] = _build_program(mind)
    return _cache[key]


# ------------------------------------------------------------------
# entry point
# ------------------------------------------------------------------
def kernel(dwi, mask, design_matrix_inv, min_diffusivity):
    dwi = np.ascontiguousarray(np.asarray(dwi, dtype=np.float32)).reshape(V_TOT, G)
    mask = np.asarray(mask, dtype=np.float32).reshape(V_TOT)
    mind = float(np.asarray(min_diffusivity))

    nc = _get_program(mind)
    wpat = _wpat(design_matrix_inv)

    # clamp (reference semantics; also keeps the DVE bit-trick log in its
    # valid domain: positive normal bf16)
    dwi_bf = np.maximum(dwi, np.float32(mind)).astype(BF16_NP).view(np.uint16)
    in_maps = []
    for core in range(NCORES):
        pad = np.empty((V_PAD, G), dtype=np.uint16)
        pad[:V_CORE] = dwi_bf[core * V_CORE : (core + 1) * V_CORE]
        pad[V_CORE:] = np.float32(1.0).astype(BF16_NP).view(np.uint16)
        in_maps.append(
            {"dwi": _pack_core(pad).view(BF16_NP), "wpat": wpat}
        )

    res = None
    for attempt in range(3):
        try:
            res = bass_utils.run_bass_kernel_spmd(nc, in_maps, core_ids=list(range(NCORES)))
            break
        except Exception:
            if attempt == 2:
                raise
    _cache["last_result"] = res  # exec_time_ns etc. for the dev harness

    vmap_flat = _voxel_map().reshape(-1)  # [P*NCH*2]
    fa = np.empty(V_TOT, dtype=np.float32)
    for core in range(NCORES):
        fa_dev = np.asarray(res.results[core]["fa"]).astype(np.float32).reshape(-1)
        fa_pad = np.empty(V_PAD, dtype=np.float32)
        fa_pad[vmap_flat] = fa_dev
        fa[core * V_CORE : (core + 1) * V_CORE] = fa_pad[:V_CORE]

    fa *= mask
    return fa.reshape(NX, NY, NZ, 1)
